# revision 13
# baseline (speedup 1.0000x reference)
"""Trainium2 Bass kernel for nn_AsynBaseStem (sparse 7x7 conv + BN + ReLU +
scatter + 3x3/2 maxpool), 8-core data-parallel over output row bands.

Architecture (per core, fully dense, no indirect DMA):
  - Host prebuilds a [128, 81*646] bf16 operand table T6 per core:
      rows 0..125  : (j,i,ch) j<6 -> fm_pad[r+i, c+j, ch]  (column-shifted planar stripes)
      row  126     : inactive flag (1.0 where pixel has no site, else 0.0)
      row  127     : ones (bias row)
  - Dense conv at every pixel via 2 accumulating matmuls (K=128 main + K=21
    tail read from T6 rows 0..20 at col offset +6). The flag row adds -1e9 to
    inactive pixels (masking), the ones row adds the BN bias.
  - PSUM eviction fuses the column max-pool (DVE even/odd max + ACT third-col
    copy), then a row ring-buffer completes the 3x3/2 max pool.
  - Final ReLU, PE-transpose to [pq, ch], DMA out as f32.

kernel(**inputs) takes FULL unsharded inputs, returns [319, 319, 64] f32.
"""
import numpy as np
import ml_dtypes
from contextlib import ExitStack

H = W = 640
CIN, COUT = 3, 64
K, PAD = 7, 3
NCORES = 8
BROWS = 81            # dense rows per core band
WPAD = W + 2 * PAD    # 646
NB = BROWS * WPAD     # T6 free size per core
PROWS = 40            # pooled rows per core (core 7: 39 valid)
QCOLS = 319
BN_EPS = 1e-5
NEG = -1.0e9


def _build_bass():
    import concourse.bass as bass
    import concourse.mybir as mybir
    import concourse.tile as tile
    from concourse import bacc

    fp32 = mybir.dt.float32
    bf16 = mybir.dt.bfloat16

    nc = bacc.Bacc()
    t6_ext = nc.declare_dram_parameter("t6", [128, NB], bf16, isOutput=False)
    w_ext = nc.declare_dram_parameter("w", [128, COUT], fp32, isOutput=False)
    wt_ext = nc.declare_dram_parameter("wtail", [21, COUT], fp32, isOutput=False)
    s126_ext = nc.declare_dram_parameter("sel126", [128, COUT], fp32, isOutput=False)
    s127_ext = nc.declare_dram_parameter("sel127", [128, COUT], fp32, isOutput=False)
    gam_ext = nc.declare_dram_parameter("gam", [128, COUT], fp32, isOutput=False)
    bet_ext = nc.declare_dram_parameter("bet", [128, COUT], fp32, isOutput=False)
    mu_ext = nc.declare_dram_parameter("mu", [128, COUT], fp32, isOutput=False)
    var_ext = nc.declare_dram_parameter("var", [128, COUT], fp32, isOutput=False)
    out_ext = nc.declare_dram_parameter("out", [PROWS, QCOLS, COUT], fp32, isOutput=True)

    with ExitStack() as ctx:
        tc = ctx.enter_context(tile.TileContext(nc))
        cpool = ctx.enter_context(tc.tile_pool(name="const", bufs=1))
        rowp = ctx.enter_context(tc.tile_pool(name="rows", bufs=6))
        ringp = ctx.enter_context(tc.tile_pool(name="ring", bufs=1))
        psp = ctx.enter_context(tc.tile_pool(name="ps", bufs=5, space="PSUM"))
        tpsp = ctx.enter_context(tc.tile_pool(name="tps", bufs=2, space="PSUM"))
        outp = ctx.enter_context(tc.tile_pool(name="outs", bufs=4))

        # ---- big operand table ----
        t6 = cpool.tile([128, NB], bf16)
        nc.sync.dma_start(t6[:], t6_ext[:])

        # ---- weight prep: lhsT A [128, 64] (W'[0:126] + flag row + bias row),
        #      lhsT B [21, 64] (W'[126:147]); W' = W * inv, inv = gamma*rsqrt(var+eps)
        wa_f = cpool.tile([128, COUT], fp32)
        nc.sync.dma_start(wa_f[:], w_ext[:])
        wb_f = cpool.tile([21, COUT], fp32)
        nc.sync.dma_start(wb_f[:], wt_ext[:])
        s126 = cpool.tile([128, COUT], fp32)
        nc.sync.dma_start(s126[:], s126_ext[:])
        s127 = cpool.tile([128, COUT], fp32)
        nc.sync.dma_start(s127[:], s127_ext[:])
        gam = cpool.tile([128, COUT], fp32)
        nc.sync.dma_start(gam[:], gam_ext[:])
        bet = cpool.tile([128, COUT], fp32)
        nc.sync.dma_start(bet[:], bet_ext[:])
        mu = cpool.tile([128, COUT], fp32)
        nc.sync.dma_start(mu[:], mu_ext[:])
        var = cpool.tile([128, COUT], fp32)
        nc.sync.dma_start(var[:], var_ext[:])

        inv = cpool.tile([128, COUT], fp32)
        nc.vector.tensor_scalar_add(inv[:], var[:], BN_EPS)
        nc.scalar.activation(inv[:], inv[:], mybir.ActivationFunctionType.Sqrt)
        nc.vector.reciprocal(inv[:], inv[:])
        nc.vector.tensor_mul(inv[:], inv[:], gam[:])
        # bias' = beta - mu*inv ; flagC = NEG - bias'
        biasv = cpool.tile([128, COUT], fp32)
        nc.vector.tensor_mul(biasv[:], mu[:], inv[:])
        nc.vector.tensor_sub(biasv[:], bet[:], biasv[:])
        flagc = cpool.tile([128, COUT], fp32)
        nc.vector.tensor_scalar(
            out=flagc[:], in0=biasv[:], scalar1=-1.0, scalar2=NEG,
            op0=mybir.AluOpType.mult, op1=mybir.AluOpType.add,
        )

        # lhsA = wa*inv (rows 126/127 are zero in wa) + sel126*flagC + sel127*bias'
        acc = cpool.tile([128, COUT], fp32)
        nc.vector.tensor_mul(acc[:], wa_f[:], inv[:])
        t1 = cpool.tile([128, COUT], fp32)
        nc.vector.tensor_mul(t1[:], s126[:], flagc[:])
        nc.vector.tensor_add(acc[:], acc[:], t1[:])
        nc.vector.tensor_mul(t1[:], s127[:], biasv[:])
        lhsA = cpool.tile([128, COUT], bf16)
        nc.vector.tensor_add(lhsA[:], acc[:], t1[:])
        lhsB = cpool.tile([21, COUT], bf16)
        nc.vector.tensor_mul(lhsB[:], wb_f[:], inv[0:21, :])

        identity = cpool.tile([64, 64], bf16)
        from concourse.masks import make_identity
        make_identity(nc, identity[:])

        # ---- pooled accumulator [64, PROWS, 320] bf16 and row ring ----
        pooled = ringp.tile([COUT, PROWS * 320], bf16)
        mring = ringp.tile([COUT, 4 * 320], bf16)  # m rows modulo 4

        # halves: (c0, width, qbase, nq)
        halves = ((0, 322, 0, 160), (320, 320, 160, 159))

        for r in range(BROWS):
            mrow = mring[:, (r % 4) * 320:(r % 4) * 320 + 320]
            for c0, wdt, qb, nq in halves:
                ps = psp.tile([COUT, 322], fp32, tag="convps")
                x = r * WPAD + c0
                nc.tensor.matmul(ps[:, 0:wdt], lhsA[:], t6[0:128, x:x + wdt],
                                 start=True, stop=False)
                nc.tensor.matmul(ps[:, 0:wdt], lhsB[:], t6[0:21, x + 6:x + 6 + wdt],
                                 start=False, stop=True)
                # col-pool m[q] = max(d[2q], d[2q+1], d[2q+2]):
                # ACT stages even cols (ev[q] = d[2q], also provides d[2q+2] = ev[q+1]);
                # DVE: t = max(ev, d-odd[psum]); m = max(t, ev-shifted)
                ev = rowp.tile([COUT, 161], bf16, tag="ev")
                nc.scalar.copy(ev[:, 0:nq + 1], ps[:, 0:2 * (nq + 1):2])
                trow = rowp.tile([COUT, 161], bf16, tag="trow")
                nc.vector.tensor_tensor(
                    out=trow[:, 0:nq],
                    in0=ev[:, 0:nq], in1=ps[:, 1:2 * nq:2],
                    op=mybir.AluOpType.max)
                nc.vector.tensor_tensor(
                    out=mrow[:, qb:qb + nq], in0=trow[:, 0:nq], in1=ev[:, 1:nq + 1],
                    op=mybir.AluOpType.max)
            # row pool: after m[2p+2] lands, pooled[p] = max(m[2p], m[2p+1], m[2p+2])
            if r >= 2 and r % 2 == 0:
                p = (r - 2) // 2
                m0 = mring[:, ((r - 2) % 4) * 320:((r - 2) % 4) * 320 + 320]
                m1 = mring[:, ((r - 1) % 4) * 320:((r - 1) % 4) * 320 + 320]
                m2 = mring[:, (r % 4) * 320:(r % 4) * 320 + 320]
                s01 = rowp.tile([COUT, 320], bf16, tag="s01")
                nc.vector.tensor_tensor(out=s01[:], in0=m0[:], in1=m1[:],
                                        op=mybir.AluOpType.max)
                po = pooled[:, p * 320:(p + 1) * 320]
                nc.vector.tensor_tensor(out=po[:], in0=s01[:], in1=m2[:],
                                        op=mybir.AluOpType.max)

        # ---- final relu + transpose to [pq, ch] + out DMA ----
        for p in range(PROWS):
            po = pooled[:, p * 320:(p + 1) * 320]
            nc.vector.tensor_scalar_max(po[:], po[:], 0.0)
            stage = outp.tile([128, 3 * COUT], fp32, tag="stage")
            for ci, qn in ((0, 128), (1, 128), (2, 63)):
                tps = tpsp.tile([128, COUT], bf16, tag="tp")
                nc.tensor.transpose(tps[0:qn, :], po[:, ci * 128:ci * 128 + qn],
                                    identity[:])
                nc.scalar.copy(stage[0:qn, ci * COUT:(ci + 1) * COUT], tps[0:qn, :])
            # out[p, q, ch]: q = ci*128 + qp  ->  src stage[qp, ci, ch]
            nc.sync.dma_start(
                out_ext[p, 0:256, :].rearrange("(c q) o -> q c o", q=128),
                stage[:].rearrange("q (c o) -> q c o", o=COUT)[:, 0:2, :])
            nc.sync.dma_start(
                out_ext[p, 256:QCOLS, :],
                stage[0:63, 2 * COUT:3 * COUT])

    nc.finalize()
    return nc


_NC_CACHE = None


def _get_nc():
    global _NC_CACHE
    if _NC_CACHE is None:
        _NC_CACHE = _build_bass()
    return _NC_CACHE


def build_in_maps(update_location, feature_map, weight, gamma, beta,
                  running_mean, running_var):
    fm = np.asarray(feature_map, np.float32)
    loc = np.asarray(update_location).astype(np.int64)
    wt = np.asarray(weight, np.float32)

    fm_pad = np.pad(fm, ((PAD, PAD), (PAD, PAD), (0, 0)))          # [646,646,3]
    # stripes B_T[t=(i,ch), r, c] = fm_pad[r+i, c, ch], r in 0..640 (row 640 pad)
    bt = np.zeros((21, H + 1, WPAD), np.float32)
    for i in range(K):
        for ch in range(CIN):
            bt[i * CIN + ch, 0:H, :] = fm_pad[i:i + H, :, ch]
    bt = bt.astype(ml_dtypes.bfloat16)

    # inactive flag = 1 where no site; indexed by output pixel (r, c) at
    # position c in the 646-pitch row; columns 640..645 stay inactive.
    flag = np.ones((H + 1, WPAD), np.float32)
    flag[loc[:, 0], loc[:, 1]] = 0.0
    flag[:, H:] = 1.0
    flag = flag.astype(ml_dtypes.bfloat16)

    # reordered weights W_re[(j,i,ch), o] = weight[i, j, ch, o]
    w_re = np.ascontiguousarray(
        wt.transpose(1, 0, 2, 3).reshape(147, COUT)).astype(np.float32)

    bcast = lambda v: np.ascontiguousarray(
        np.broadcast_to(np.asarray(v, np.float32)[None, :], (128, COUT)))

    in_maps = []
    for k in range(NCORES):
        r0 = 80 * k
        t6 = np.zeros((128, BROWS, WPAD), ml_dtypes.bfloat16)
        for j in range(6):
            sl = bt[:, r0:r0 + BROWS, :]
            t6[j * 21:(j + 1) * 21, :, :-j or None] = sl[:, :, j:]
        t6[126] = flag[r0:r0 + BROWS]
        t6[127] = np.ones((BROWS, WPAD), ml_dtypes.bfloat16)
        wfull = np.zeros((128, COUT), np.float32)
        wfull[0:126] = w_re[0:126]
        sel126 = np.zeros((128, COUT), np.float32)
        sel126[126] = 1.0
        sel127 = np.zeros((128, COUT), np.float32)
        sel127[127] = 1.0
        in_maps.append({
            "t6": np.ascontiguousarray(t6.reshape(128, NB)),
            "w": wfull,
            "wtail": np.ascontiguousarray(w_re[126:147]),
            "sel126": sel126, "sel127": sel127,
            "gam": bcast(gamma), "bet": bcast(beta),
            "mu": bcast(running_mean), "var": bcast(running_var),
        })
    return in_maps


def kernel(update_location, feature_map, weight, gamma, beta, running_mean,
           running_var):
    from concourse.bass_utils import run_bass_kernel_spmd

    in_maps = build_in_maps(update_location, feature_map, weight, gamma, beta,
                            running_mean, running_var)
    nc = _get_nc()
    res = run_bass_kernel_spmd(nc, in_maps, core_ids=list(range(NCORES)))
    out = np.concatenate([res.results[k]["out"] for k in range(NCORES)], axis=0)
    return np.ascontiguousarray(out[:QCOLS]).astype(np.float32)


# revision 16
# speedup vs baseline: 1.2823x; 1.2823x over previous
"""Trainium2 Bass kernel for nn_AsynBaseStem (sparse 7x7 conv + BN + ReLU +
scatter + 3x3/2 maxpool), 8-core data-parallel over output row bands.

Architecture (per core, fully dense, no indirect DMA):
  - Host prebuilds a [128, 81*646] bf16 operand table T6 per core:
      rows 0..125  : (j,i,ch) j<6 -> fm_pad[r+i, c+j, ch]  (column-shifted planar stripes)
      row  126     : inactive flag (1.0 where pixel has no site, else 0.0)
      row  127     : ones (bias row)
  - Dense conv at every pixel via 2 accumulating matmuls (K=128 main + K=21
    tail read from T6 rows 0..20 at col offset +6). The flag row adds -1e9 to
    inactive pixels (masking), the ones row adds the BN bias.
  - PSUM eviction fuses the column max-pool (DVE even/odd max + ACT third-col
    copy), then a row ring-buffer completes the 3x3/2 max pool.
  - Final ReLU, PE-transpose to [pq, ch], DMA out as f32.

kernel(**inputs) takes FULL unsharded inputs, returns [319, 319, 64] f32.
"""
import numpy as np
import ml_dtypes
from contextlib import ExitStack

H = W = 640
CIN, COUT = 3, 64
K, PAD = 7, 3
NCORES = 8
BROWS = 81            # dense rows per core band
WPAD = W + 2 * PAD    # 646
NB = BROWS * WPAD     # T6 free size per core
PROWS = 40            # pooled rows per core (core 7: 39 valid)
QCOLS = 319
BN_EPS = 1e-5
NEG = -1.0e9


def _build_bass():
    import concourse.bass as bass
    import concourse.mybir as mybir
    import concourse.tile as tile
    from concourse import bacc

    fp32 = mybir.dt.float32
    bf16 = mybir.dt.bfloat16

    nc = bacc.Bacc()
    t6_ext = nc.declare_dram_parameter("t6", [128, NB], bf16, isOutput=False)
    w_ext = nc.declare_dram_parameter("w", [128, COUT], fp32, isOutput=False)
    wt_ext = nc.declare_dram_parameter("wtail", [21, COUT], fp32, isOutput=False)
    s126_ext = nc.declare_dram_parameter("sel126", [128, COUT], fp32, isOutput=False)
    s127_ext = nc.declare_dram_parameter("sel127", [128, COUT], fp32, isOutput=False)
    gam_ext = nc.declare_dram_parameter("gam", [128, COUT], fp32, isOutput=False)
    bet_ext = nc.declare_dram_parameter("bet", [128, COUT], fp32, isOutput=False)
    mu_ext = nc.declare_dram_parameter("mu", [128, COUT], fp32, isOutput=False)
    var_ext = nc.declare_dram_parameter("var", [128, COUT], fp32, isOutput=False)
    out_ext = nc.declare_dram_parameter("out", [COUT, PROWS * 320], fp32, isOutput=True)

    with ExitStack() as ctx:
        tc = ctx.enter_context(tile.TileContext(nc))
        cpool = ctx.enter_context(tc.tile_pool(name="const", bufs=1))
        rowp = ctx.enter_context(tc.tile_pool(name="rows", bufs=6))
        ringp = ctx.enter_context(tc.tile_pool(name="ring", bufs=1))
        psp = ctx.enter_context(tc.tile_pool(name="ps", bufs=8, space="PSUM"))

        # ---- big operand table ----
        t6 = cpool.tile([128, NB], bf16)
        nc.sync.dma_start(t6[:], t6_ext[:])

        # ---- weight prep: lhsT A [128, 64] (W'[0:126] + flag row + bias row),
        #      lhsT B [21, 64] (W'[126:147]); W' = W * inv, inv = gamma*rsqrt(var+eps)
        wa_f = cpool.tile([128, COUT], fp32)
        nc.sync.dma_start(wa_f[:], w_ext[:])
        wb_f = cpool.tile([21, COUT], fp32)
        nc.sync.dma_start(wb_f[:], wt_ext[:])
        s126 = cpool.tile([128, COUT], fp32)
        nc.sync.dma_start(s126[:], s126_ext[:])
        s127 = cpool.tile([128, COUT], fp32)
        nc.sync.dma_start(s127[:], s127_ext[:])
        gam = cpool.tile([128, COUT], fp32)
        nc.sync.dma_start(gam[:], gam_ext[:])
        bet = cpool.tile([128, COUT], fp32)
        nc.sync.dma_start(bet[:], bet_ext[:])
        mu = cpool.tile([128, COUT], fp32)
        nc.sync.dma_start(mu[:], mu_ext[:])
        var = cpool.tile([128, COUT], fp32)
        nc.sync.dma_start(var[:], var_ext[:])

        inv = cpool.tile([128, COUT], fp32)
        nc.vector.tensor_scalar_add(inv[:], var[:], BN_EPS)
        nc.scalar.activation(inv[:], inv[:], mybir.ActivationFunctionType.Sqrt)
        nc.vector.reciprocal(inv[:], inv[:])
        nc.vector.tensor_mul(inv[:], inv[:], gam[:])
        # bias' = beta - mu*inv ; flagC = NEG - bias'
        biasv = cpool.tile([128, COUT], fp32)
        nc.vector.tensor_mul(biasv[:], mu[:], inv[:])
        nc.vector.tensor_sub(biasv[:], bet[:], biasv[:])
        flagc = cpool.tile([128, COUT], fp32)
        nc.vector.tensor_scalar(
            out=flagc[:], in0=biasv[:], scalar1=-1.0, scalar2=NEG,
            op0=mybir.AluOpType.mult, op1=mybir.AluOpType.add,
        )

        # lhsA = wa*inv (rows 126/127 are zero in wa) + sel126*flagC + sel127*bias'
        acc = cpool.tile([128, COUT], fp32)
        nc.vector.tensor_mul(acc[:], wa_f[:], inv[:])
        t1 = cpool.tile([128, COUT], fp32)
        nc.vector.tensor_mul(t1[:], s126[:], flagc[:])
        nc.vector.tensor_add(acc[:], acc[:], t1[:])
        nc.vector.tensor_mul(t1[:], s127[:], biasv[:])
        lhsA = cpool.tile([128, COUT], bf16)
        nc.vector.tensor_add(lhsA[:], acc[:], t1[:])
        lhsB = cpool.tile([21, COUT], bf16)
        nc.vector.tensor_mul(lhsB[:], wb_f[:], inv[0:21, :])

        # ---- pooled accumulator [64, PROWS, 320] bf16 and row ring ----
        pooled = ringp.tile([COUT, PROWS * 320], bf16)
        mring = ringp.tile([COUT, 4 * 320], bf16)  # m rows modulo 4

        # halves: (c0, width, qbase, nq)
        halves = ((0, 322, 0, 160), (320, 320, 160, 159))

        for r in range(BROWS):
            mrow = mring[:, (r % 4) * 320:(r % 4) * 320 + 320]
            for c0, wdt, qb, nq in halves:
                ps = psp.tile([COUT, 322], fp32, tag="convps")
                x = r * WPAD + c0
                nc.tensor.matmul(ps[:, 0:wdt], lhsA[:], t6[0:128, x:x + wdt],
                                 start=True, stop=False)
                nc.tensor.matmul(ps[:, 0:wdt], lhsB[:], t6[0:21, x + 6:x + 6 + wdt],
                                 start=False, stop=True)
                # col-pool m[q] = max(d[2q], d[2q+1], d[2q+2]):
                # ACT stages even cols (ev[q] = d[2q], also provides d[2q+2] = ev[q+1]);
                # DVE: t = max(ev, d-odd[psum]); m = max(t, ev-shifted)
                ev = rowp.tile([COUT, 161], bf16, tag="ev")
                nc.scalar.copy(ev[:, 0:nq + 1], ps[:, 0:2 * (nq + 1):2])
                trow = rowp.tile([COUT, 161], bf16, tag="trow")
                nc.vector.tensor_tensor(
                    out=trow[:, 0:nq],
                    in0=ev[:, 0:nq], in1=ps[:, 1:2 * nq:2],
                    op=mybir.AluOpType.max)
                nc.vector.tensor_tensor(
                    out=mrow[:, qb:qb + nq], in0=trow[:, 0:nq], in1=ev[:, 1:nq + 1],
                    op=mybir.AluOpType.max)
            # row pool (+final relu): pooled[p] = max(m[2p], m[2p+1], m[2p+2], 0)
            if r >= 2 and r % 2 == 0:
                p = (r - 2) // 2
                m0 = mring[:, ((r - 2) % 4) * 320:((r - 2) % 4) * 320 + 320]
                m1 = mring[:, ((r - 1) % 4) * 320:((r - 1) % 4) * 320 + 320]
                m2 = mring[:, (r % 4) * 320:(r % 4) * 320 + 320]
                s01 = rowp.tile([COUT, 320], bf16, tag="s01")
                nc.vector.tensor_tensor(out=s01[:], in0=m0[:], in1=m1[:],
                                        op=mybir.AluOpType.max)
                po = pooled[:, p * 320:(p + 1) * 320]
                nc.vector.scalar_tensor_tensor(
                    out=po[:], in0=s01[:], scalar=0.0, in1=m2[:],
                    op0=mybir.AluOpType.max, op1=mybir.AluOpType.max)

        # ---- out DMA (cast bf16 -> f32); host does final [p,q,ch] transpose ----
        nc.gpsimd.dma_start(out_ext[:], pooled[:])

    nc.finalize()
    return nc


_NC_CACHE = None


def _get_nc():
    global _NC_CACHE
    if _NC_CACHE is None:
        _NC_CACHE = _build_bass()
    return _NC_CACHE


def build_in_maps(update_location, feature_map, weight, gamma, beta,
                  running_mean, running_var):
    fm = np.asarray(feature_map, np.float32)
    loc = np.asarray(update_location).astype(np.int64)
    wt = np.asarray(weight, np.float32)

    fm_pad = np.pad(fm, ((PAD, PAD), (PAD, PAD), (0, 0)))          # [646,646,3]
    # stripes B_T[t=(i,ch), r, c] = fm_pad[r+i, c, ch], r in 0..640 (row 640 pad)
    bt = np.zeros((21, H + 1, WPAD), np.float32)
    for i in range(K):
        for ch in range(CIN):
            bt[i * CIN + ch, 0:H, :] = fm_pad[i:i + H, :, ch]
    bt = bt.astype(ml_dtypes.bfloat16)

    # inactive flag = 1 where no site; indexed by output pixel (r, c) at
    # position c in the 646-pitch row; columns 640..645 stay inactive.
    flag = np.ones((H + 1, WPAD), np.float32)
    flag[loc[:, 0], loc[:, 1]] = 0.0
    flag[:, H:] = 1.0
    flag = flag.astype(ml_dtypes.bfloat16)

    # reordered weights W_re[(j,i,ch), o] = weight[i, j, ch, o]
    w_re = np.ascontiguousarray(
        wt.transpose(1, 0, 2, 3).reshape(147, COUT)).astype(np.float32)

    bcast = lambda v: np.ascontiguousarray(
        np.broadcast_to(np.asarray(v, np.float32)[None, :], (128, COUT)))

    in_maps = []
    for k in range(NCORES):
        r0 = 80 * k
        t6 = np.zeros((128, BROWS, WPAD), ml_dtypes.bfloat16)
        for j in range(6):
            sl = bt[:, r0:r0 + BROWS, :]
            t6[j * 21:(j + 1) * 21, :, :-j or None] = sl[:, :, j:]
        t6[126] = flag[r0:r0 + BROWS]
        t6[127] = np.ones((BROWS, WPAD), ml_dtypes.bfloat16)
        wfull = np.zeros((128, COUT), np.float32)
        wfull[0:126] = w_re[0:126]
        sel126 = np.zeros((128, COUT), np.float32)
        sel126[126] = 1.0
        sel127 = np.zeros((128, COUT), np.float32)
        sel127[127] = 1.0
        in_maps.append({
            "t6": np.ascontiguousarray(t6.reshape(128, NB)),
            "w": wfull,
            "wtail": np.ascontiguousarray(w_re[126:147]),
            "sel126": sel126, "sel127": sel127,
            "gam": bcast(gamma), "bet": bcast(beta),
            "mu": bcast(running_mean), "var": bcast(running_var),
        })
    return in_maps


def kernel(update_location, feature_map, weight, gamma, beta, running_mean,
           running_var):
    from concourse.bass_utils import run_bass_kernel_spmd

    in_maps = build_in_maps(update_location, feature_map, weight, gamma, beta,
                            running_mean, running_var)
    nc = _get_nc()
    res = run_bass_kernel_spmd(nc, in_maps, core_ids=list(range(NCORES)))
    # per-core out is [64, PROWS*320] f32 (ch-major); assemble [319, 319, 64]
    parts = []
    for k in range(NCORES):
        o = res.results[k]["out"].reshape(COUT, PROWS, 320)
        parts.append(o.transpose(1, 2, 0)[:, :QCOLS, :])
    out = np.concatenate(parts, axis=0)[:QCOLS]
    return np.ascontiguousarray(out).astype(np.float32)


# revision 17
# speedup vs baseline: 1.3402x; 1.0451x over previous
"""Trainium2 Bass kernel for nn_AsynBaseStem (sparse 7x7 conv + BN + ReLU +
scatter + 3x3/2 maxpool), 8-core data-parallel over output row bands.

Architecture (per core, fully dense, no indirect DMA):
  - Host prebuilds a [128, 81*646] bf16 operand table T6 per core:
      rows 0..125  : (j,i,ch) j<6 -> fm_pad[r+i, c+j, ch]  (column-shifted planar stripes)
      row  126     : inactive flag (1.0 where pixel has no site, else 0.0)
      row  127     : ones (bias row)
  - Dense conv at every pixel via 2 accumulating matmuls (K=128 main + K=21
    tail read from T6 rows 0..20 at col offset +6). The flag row adds -1e9 to
    inactive pixels (masking), the ones row adds the BN bias.
  - PSUM eviction fuses the column max-pool (DVE even/odd max + ACT third-col
    copy), then a row ring-buffer completes the 3x3/2 max pool.
  - Final ReLU folded into the row pool; one cast-DMA writes [64, p*320] f32;
    the host transposes to [p, q, ch] during unsharding.

kernel(**inputs) takes FULL unsharded inputs, returns [319, 319, 64] f32.
"""
import numpy as np
import ml_dtypes
from contextlib import ExitStack

H = W = 640
CIN, COUT = 3, 64
K, PAD = 7, 3
NCORES = 8
BROWS = 81            # dense rows per core band
WPAD = W + 2 * PAD    # 646
NB = BROWS * WPAD     # T6 free size per core
PROWS = 40            # pooled rows per core (core 7: 39 valid)
QCOLS = 319
BN_EPS = 1e-5
NEG = -1.0e9


def _build_bass():
    import concourse.bass as bass
    import concourse.mybir as mybir
    import concourse.tile as tile
    from concourse import bacc

    fp32 = mybir.dt.float32
    bf16 = mybir.dt.bfloat16

    nc = bacc.Bacc()
    t6_ext = nc.declare_dram_parameter("t6", [128, NB], bf16, isOutput=False)
    w_ext = nc.declare_dram_parameter("w", [128, COUT], fp32, isOutput=False)
    wt_ext = nc.declare_dram_parameter("wtail", [21, COUT], fp32, isOutput=False)
    s126_ext = nc.declare_dram_parameter("sel126", [128, COUT], fp32, isOutput=False)
    s127_ext = nc.declare_dram_parameter("sel127", [128, COUT], fp32, isOutput=False)
    gam_ext = nc.declare_dram_parameter("gam", [128, COUT], fp32, isOutput=False)
    bet_ext = nc.declare_dram_parameter("bet", [128, COUT], fp32, isOutput=False)
    mu_ext = nc.declare_dram_parameter("mu", [128, COUT], fp32, isOutput=False)
    var_ext = nc.declare_dram_parameter("var", [128, COUT], fp32, isOutput=False)
    out_ext = nc.declare_dram_parameter("out", [COUT, PROWS * 320], fp32, isOutput=True)

    with ExitStack() as ctx:
        tc = ctx.enter_context(tile.TileContext(nc))
        cpool = ctx.enter_context(tc.tile_pool(name="const", bufs=1))
        rowp = ctx.enter_context(tc.tile_pool(name="rows", bufs=12))
        ringp = ctx.enter_context(tc.tile_pool(name="ring", bufs=1))
        psp = ctx.enter_context(tc.tile_pool(name="ps", bufs=8, space="PSUM"))

        # ---- big operand table ----
        t6 = cpool.tile([128, NB], bf16)
        nc.sync.dma_start(t6[:], t6_ext[:])

        # ---- weight prep: lhsT A [128, 64] (W'[0:126] + flag row + bias row),
        #      lhsT B [21, 64] (W'[126:147]); W' = W * inv, inv = gamma*rsqrt(var+eps)
        wa_f = cpool.tile([128, COUT], fp32)
        nc.sync.dma_start(wa_f[:], w_ext[:])
        wb_f = cpool.tile([21, COUT], fp32)
        nc.sync.dma_start(wb_f[:], wt_ext[:])
        s126 = cpool.tile([128, COUT], fp32)
        nc.sync.dma_start(s126[:], s126_ext[:])
        s127 = cpool.tile([128, COUT], fp32)
        nc.sync.dma_start(s127[:], s127_ext[:])
        gam = cpool.tile([128, COUT], fp32)
        nc.sync.dma_start(gam[:], gam_ext[:])
        bet = cpool.tile([128, COUT], fp32)
        nc.sync.dma_start(bet[:], bet_ext[:])
        mu = cpool.tile([128, COUT], fp32)
        nc.sync.dma_start(mu[:], mu_ext[:])
        var = cpool.tile([128, COUT], fp32)
        nc.sync.dma_start(var[:], var_ext[:])

        inv = cpool.tile([128, COUT], fp32)
        nc.vector.tensor_scalar_add(inv[:], var[:], BN_EPS)
        nc.scalar.activation(inv[:], inv[:], mybir.ActivationFunctionType.Sqrt)
        nc.vector.reciprocal(inv[:], inv[:])
        nc.vector.tensor_mul(inv[:], inv[:], gam[:])
        # bias' = beta - mu*inv ; flagC = NEG - bias'
        biasv = cpool.tile([128, COUT], fp32)
        nc.vector.tensor_mul(biasv[:], mu[:], inv[:])
        nc.vector.tensor_sub(biasv[:], bet[:], biasv[:])
        flagc = cpool.tile([128, COUT], fp32)
        nc.vector.tensor_scalar(
            out=flagc[:], in0=biasv[:], scalar1=-1.0, scalar2=NEG,
            op0=mybir.AluOpType.mult, op1=mybir.AluOpType.add,
        )

        # lhsA = wa*inv (rows 126/127 are zero in wa) + sel126*flagC + sel127*bias'
        acc = cpool.tile([128, COUT], fp32)
        nc.vector.tensor_mul(acc[:], wa_f[:], inv[:])
        t1 = cpool.tile([128, COUT], fp32)
        nc.vector.tensor_mul(t1[:], s126[:], flagc[:])
        nc.vector.tensor_add(acc[:], acc[:], t1[:])
        nc.vector.tensor_mul(t1[:], s127[:], biasv[:])
        lhsA = cpool.tile([128, COUT], bf16)
        nc.vector.tensor_add(lhsA[:], acc[:], t1[:])
        lhsB = cpool.tile([21, COUT], bf16)
        nc.vector.tensor_mul(lhsB[:], wb_f[:], inv[0:21, :])

        # ---- pooled accumulator [64, PROWS, 320] bf16 and row ring ----
        pooled = ringp.tile([COUT, PROWS * 320], bf16)
        mring = ringp.tile([COUT, 8 * 320], bf16)  # m rows modulo 8

        # halves: (c0, width, qbase, nq)
        halves = ((0, 322, 0, 160), (320, 320, 160, 159))

        for r in range(BROWS):
            mrow = mring[:, (r % 8) * 320:(r % 8) * 320 + 320]
            for c0, wdt, qb, nq in halves:
                ps = psp.tile([COUT, 322], fp32, tag="convps")
                x = r * WPAD + c0
                nc.tensor.matmul(ps[:, 0:wdt], lhsA[:], t6[0:128, x:x + wdt],
                                 start=True, stop=False)
                nc.tensor.matmul(ps[:, 0:wdt], lhsB[:], t6[0:21, x + 6:x + 6 + wdt],
                                 start=False, stop=True)
                # col-pool m[q] = max(d[2q], d[2q+1], d[2q+2]):
                # ACT stages even cols (ev[q] = d[2q], also provides d[2q+2] = ev[q+1]);
                # DVE: t = max(ev, d-odd[psum]); m = max(t, ev-shifted)
                ev = rowp.tile([COUT, 161], bf16, tag="ev")
                nc.scalar.copy(ev[:, 0:nq + 1], ps[:, 0:2 * (nq + 1):2])
                trow = rowp.tile([COUT, 161], bf16, tag="trow")
                nc.vector.tensor_tensor(
                    out=trow[:, 0:nq],
                    in0=ev[:, 0:nq], in1=ps[:, 1:2 * nq:2],
                    op=mybir.AluOpType.max)
                nc.vector.tensor_tensor(
                    out=mrow[:, qb:qb + nq], in0=trow[:, 0:nq], in1=ev[:, 1:nq + 1],
                    op=mybir.AluOpType.max)
            # row pool (+final relu): pooled[p] = max(m[2p], m[2p+1], m[2p+2], 0)
            if r >= 2 and r % 2 == 0:
                p = (r - 2) // 2
                m0 = mring[:, ((r - 2) % 8) * 320:((r - 2) % 8) * 320 + 320]
                m1 = mring[:, ((r - 1) % 8) * 320:((r - 1) % 8) * 320 + 320]
                m2 = mring[:, (r % 8) * 320:(r % 8) * 320 + 320]
                s01 = rowp.tile([COUT, 320], bf16, tag="s01")
                nc.vector.tensor_tensor(out=s01[:], in0=m0[:], in1=m1[:],
                                        op=mybir.AluOpType.max)
                po = pooled[:, p * 320:(p + 1) * 320]
                nc.vector.scalar_tensor_tensor(
                    out=po[:], in0=s01[:], scalar=0.0, in1=m2[:],
                    op0=mybir.AluOpType.max, op1=mybir.AluOpType.max)

        # ---- out DMA (cast bf16 -> f32); host does final [p,q,ch] transpose ----
        nc.gpsimd.dma_start(out_ext[:], pooled[:])

    nc.finalize()
    return nc


_NC_CACHE = None


def _get_nc():
    global _NC_CACHE
    if _NC_CACHE is None:
        _NC_CACHE = _build_bass()
    return _NC_CACHE


def build_in_maps(update_location, feature_map, weight, gamma, beta,
                  running_mean, running_var):
    fm = np.asarray(feature_map, np.float32)
    loc = np.asarray(update_location).astype(np.int64)
    wt = np.asarray(weight, np.float32)

    fm_pad = np.pad(fm, ((PAD, PAD), (PAD, PAD), (0, 0)))          # [646,646,3]
    # stripes B_T[t=(i,ch), r, c] = fm_pad[r+i, c, ch], r in 0..640 (row 640 pad)
    bt = np.zeros((21, H + 1, WPAD), np.float32)
    for i in range(K):
        for ch in range(CIN):
            bt[i * CIN + ch, 0:H, :] = fm_pad[i:i + H, :, ch]
    bt = bt.astype(ml_dtypes.bfloat16)

    # inactive flag = 1 where no site; indexed by output pixel (r, c) at
    # position c in the 646-pitch row; columns 640..645 stay inactive.
    flag = np.ones((H + 1, WPAD), np.float32)
    flag[loc[:, 0], loc[:, 1]] = 0.0
    flag[:, H:] = 1.0
    flag = flag.astype(ml_dtypes.bfloat16)

    # reordered weights W_re[(j,i,ch), o] = weight[i, j, ch, o]
    w_re = np.ascontiguousarray(
        wt.transpose(1, 0, 2, 3).reshape(147, COUT)).astype(np.float32)

    bcast = lambda v: np.ascontiguousarray(
        np.broadcast_to(np.asarray(v, np.float32)[None, :], (128, COUT)))

    in_maps = []
    for k in range(NCORES):
        r0 = 80 * k
        t6 = np.zeros((128, BROWS, WPAD), ml_dtypes.bfloat16)
        for j in range(6):
            sl = bt[:, r0:r0 + BROWS, :]
            t6[j * 21:(j + 1) * 21, :, :-j or None] = sl[:, :, j:]
        t6[126] = flag[r0:r0 + BROWS]
        t6[127] = np.ones((BROWS, WPAD), ml_dtypes.bfloat16)
        wfull = np.zeros((128, COUT), np.float32)
        wfull[0:126] = w_re[0:126]
        sel126 = np.zeros((128, COUT), np.float32)
        sel126[126] = 1.0
        sel127 = np.zeros((128, COUT), np.float32)
        sel127[127] = 1.0
        in_maps.append({
            "t6": np.ascontiguousarray(t6.reshape(128, NB)),
            "w": wfull,
            "wtail": np.ascontiguousarray(w_re[126:147]),
            "sel126": sel126, "sel127": sel127,
            "gam": bcast(gamma), "bet": bcast(beta),
            "mu": bcast(running_mean), "var": bcast(running_var),
        })
    return in_maps


def kernel(update_location, feature_map, weight, gamma, beta, running_mean,
           running_var):
    from concourse.bass_utils import run_bass_kernel_spmd

    in_maps = build_in_maps(update_location, feature_map, weight, gamma, beta,
                            running_mean, running_var)
    nc = _get_nc()
    res = run_bass_kernel_spmd(nc, in_maps, core_ids=list(range(NCORES)))
    # per-core out is [64, PROWS*320] f32 (ch-major); assemble [319, 319, 64]
    parts = []
    for k in range(NCORES):
        o = res.results[k]["out"].reshape(COUT, PROWS, 320)
        parts.append(o.transpose(1, 2, 0)[:, :QCOLS, :])
    out = np.concatenate(parts, axis=0)[:QCOLS]
    return np.ascontiguousarray(out).astype(np.float32)


# revision 20
# speedup vs baseline: 1.3771x; 1.0276x over previous
"""Trainium2 Bass kernel for nn_AsynBaseStem (sparse 7x7 conv + BN + ReLU +
scatter + 3x3/2 maxpool), 8-core data-parallel over output row bands.

Architecture (per core, fully dense, no indirect DMA):
  - Host prebuilds a [128, 81*646] bf16 operand table T6 per core:
      rows 0..125  : (j,i,ch) j<6 -> fm_pad[r+i, c+j, ch]  (column-shifted planar stripes)
      row  126     : inactive flag (1.0 where pixel has no site, else 0.0)
      row  127     : ones (bias row)
  - Dense conv at every pixel via 2 accumulating matmuls (K=128 main + K=21
    tail read from T6 rows 0..20 at col offset +6). The flag row adds -1e9 to
    inactive pixels (masking), the ones row adds the BN bias.
  - PSUM eviction fuses the column max-pool (DVE even/odd max + ACT third-col
    copy), then a row ring-buffer completes the 3x3/2 max pool.
  - Final ReLU folded into the row pool; one cast-DMA writes [64, p*320] f32;
    the host transposes to [p, q, ch] during unsharding.

kernel(**inputs) takes FULL unsharded inputs, returns [319, 319, 64] f32.
"""
import numpy as np
import ml_dtypes
from contextlib import ExitStack

H = W = 640
CIN, COUT = 3, 64
K, PAD = 7, 3
NCORES = 8
BROWS = 81            # dense rows per core band
WPAD = W + 2 * PAD    # 646
NB = BROWS * WPAD     # T6 free size per core
PROWS = 40            # pooled rows per core (core 7: 39 valid)
QCOLS = 319
BN_EPS = 1e-5
NEG = -1.0e9


def _build_bass():
    import concourse.bass as bass
    import concourse.mybir as mybir
    import concourse.tile as tile
    from concourse import bacc

    fp32 = mybir.dt.float32
    bf16 = mybir.dt.bfloat16

    nc = bacc.Bacc()
    t6_ext = nc.declare_dram_parameter("t6", [128, NB], bf16, isOutput=False)
    w_ext = nc.declare_dram_parameter("w", [128, COUT], fp32, isOutput=False)
    wt_ext = nc.declare_dram_parameter("wtail", [21, COUT], fp32, isOutput=False)
    s126_ext = nc.declare_dram_parameter("sel126", [128, COUT], fp32, isOutput=False)
    s127_ext = nc.declare_dram_parameter("sel127", [128, COUT], fp32, isOutput=False)
    gam_ext = nc.declare_dram_parameter("gam", [128, COUT], fp32, isOutput=False)
    bet_ext = nc.declare_dram_parameter("bet", [128, COUT], fp32, isOutput=False)
    mu_ext = nc.declare_dram_parameter("mu", [128, COUT], fp32, isOutput=False)
    var_ext = nc.declare_dram_parameter("var", [128, COUT], fp32, isOutput=False)
    out_ext = nc.declare_dram_parameter("out", [COUT, PROWS * 320], fp32, isOutput=True)

    with ExitStack() as ctx:
        tc = ctx.enter_context(tile.TileContext(nc))
        cpool = ctx.enter_context(tc.tile_pool(name="const", bufs=1))
        rowp = ctx.enter_context(tc.tile_pool(name="rows", bufs=12))
        ringp = ctx.enter_context(tc.tile_pool(name="ring", bufs=1))
        psp = ctx.enter_context(tc.tile_pool(name="ps", bufs=8, space="PSUM"))

        # ---- big operand table (chunked load so the conv starts early) ----
        t6 = cpool.tile([128, NB], bf16)
        for ck in range(9):
            sl = slice(ck * 9 * WPAD, (ck + 1) * 9 * WPAD)
            nc.sync.dma_start(t6[:, sl], t6_ext[:, sl])

        # ---- weight prep: lhsT A [128, 64] (W'[0:126] + flag row + bias row),
        #      lhsT B [21, 64] (W'[126:147]); W' = W * inv, inv = gamma*rsqrt(var+eps)
        wa_f = cpool.tile([128, COUT], fp32)
        nc.sync.dma_start(wa_f[:], w_ext[:])
        wb_f = cpool.tile([21, COUT], fp32)
        nc.sync.dma_start(wb_f[:], wt_ext[:])
        s126 = cpool.tile([128, COUT], fp32)
        nc.sync.dma_start(s126[:], s126_ext[:])
        s127 = cpool.tile([128, COUT], fp32)
        nc.sync.dma_start(s127[:], s127_ext[:])
        gam = cpool.tile([128, COUT], fp32)
        nc.sync.dma_start(gam[:], gam_ext[:])
        bet = cpool.tile([128, COUT], fp32)
        nc.sync.dma_start(bet[:], bet_ext[:])
        mu = cpool.tile([128, COUT], fp32)
        nc.sync.dma_start(mu[:], mu_ext[:])
        var = cpool.tile([128, COUT], fp32)
        nc.sync.dma_start(var[:], var_ext[:])

        inv = cpool.tile([128, COUT], fp32)
        nc.vector.tensor_scalar_add(inv[:], var[:], BN_EPS)
        nc.scalar.activation(inv[:], inv[:], mybir.ActivationFunctionType.Sqrt)
        nc.vector.reciprocal(inv[:], inv[:])
        nc.vector.tensor_mul(inv[:], inv[:], gam[:])
        # bias' = beta - mu*inv ; flagC = NEG - bias'
        biasv = cpool.tile([128, COUT], fp32)
        nc.vector.tensor_mul(biasv[:], mu[:], inv[:])
        nc.vector.tensor_sub(biasv[:], bet[:], biasv[:])
        flagc = cpool.tile([128, COUT], fp32)
        nc.vector.tensor_scalar(
            out=flagc[:], in0=biasv[:], scalar1=-1.0, scalar2=NEG,
            op0=mybir.AluOpType.mult, op1=mybir.AluOpType.add,
        )

        # lhsA = wa*inv (rows 126/127 are zero in wa) + sel126*flagC + sel127*bias'
        acc = cpool.tile([128, COUT], fp32)
        nc.vector.tensor_mul(acc[:], wa_f[:], inv[:])
        t1 = cpool.tile([128, COUT], fp32)
        nc.vector.tensor_mul(t1[:], s126[:], flagc[:])
        nc.vector.tensor_add(acc[:], acc[:], t1[:])
        nc.vector.tensor_mul(t1[:], s127[:], biasv[:])
        lhsA = cpool.tile([128, COUT], bf16)
        nc.vector.tensor_add(lhsA[:], acc[:], t1[:])
        lhsB = cpool.tile([21, COUT], bf16)
        nc.vector.tensor_mul(lhsB[:], wb_f[:], inv[0:21, :])

        # ---- pooled accumulator [64, PROWS, 320] bf16 and row ring ----
        pooled = ringp.tile([COUT, PROWS * 320], bf16)
        mring = ringp.tile([COUT, 8 * 320], bf16)  # m rows modulo 8

        # halves: (c0, width, qbase, nq)
        halves = ((0, 322, 0, 160), (320, 320, 160, 159))

        for r in range(BROWS):
            mrow = mring[:, (r % 8) * 320:(r % 8) * 320 + 320]
            for c0, wdt, qb, nq in halves:
                ps = psp.tile([COUT, 322], fp32, tag="convps")
                x = r * WPAD + c0
                nc.tensor.matmul(ps[:, 0:wdt], lhsA[:], t6[0:128, x:x + wdt],
                                 start=True, stop=False)
                nc.tensor.matmul(ps[:, 0:wdt], lhsB[:], t6[0:21, x + 6:x + 6 + wdt],
                                 start=False, stop=True)
                # col-pool m[q] = max(d[2q], d[2q+1], d[2q+2]):
                # ACT stages even cols (ev[q] = d[2q], also provides d[2q+2] = ev[q+1]);
                # DVE: t = max(ev, d-odd[psum]); m = max(t, ev-shifted)
                ev = rowp.tile([COUT, 161], bf16, tag="ev")
                nc.scalar.copy(ev[:, 0:nq + 1], ps[:, 0:2 * (nq + 1):2])
                trow = rowp.tile([COUT, 161], bf16, tag="trow")
                nc.vector.tensor_tensor(
                    out=trow[:, 0:nq],
                    in0=ev[:, 0:nq], in1=ps[:, 1:2 * nq:2],
                    op=mybir.AluOpType.max)
                nc.vector.tensor_tensor(
                    out=mrow[:, qb:qb + nq], in0=trow[:, 0:nq], in1=ev[:, 1:nq + 1],
                    op=mybir.AluOpType.max)
            # row pool (+final relu): pooled[p] = max(m[2p], m[2p+1], m[2p+2], 0)
            if r >= 2 and r % 2 == 0:
                p = (r - 2) // 2
                m0 = mring[:, ((r - 2) % 8) * 320:((r - 2) % 8) * 320 + 320]
                m1 = mring[:, ((r - 1) % 8) * 320:((r - 1) % 8) * 320 + 320]
                m2 = mring[:, (r % 8) * 320:(r % 8) * 320 + 320]
                s01 = rowp.tile([COUT, 320], bf16, tag="s01")
                nc.vector.tensor_tensor(out=s01[:], in0=m0[:], in1=m1[:],
                                        op=mybir.AluOpType.max)
                po = pooled[:, p * 320:(p + 1) * 320]
                nc.vector.scalar_tensor_tensor(
                    out=po[:], in0=s01[:], scalar=0.0, in1=m2[:],
                    op0=mybir.AluOpType.max, op1=mybir.AluOpType.max)
                # stream pooled rows out in chunks of 10 (cast bf16 -> f32);
                # host does the final [p,q,ch] transpose during unsharding
                if p % 10 == 9:
                    pc = p // 10
                    nc.gpsimd.dma_start(
                        out_ext[:, pc * 3200:(pc + 1) * 3200],
                        pooled[:, pc * 3200:(pc + 1) * 3200])



    nc.finalize()
    return nc


_NC_CACHE = None


def _get_nc():
    global _NC_CACHE
    if _NC_CACHE is None:
        _NC_CACHE = _build_bass()
    return _NC_CACHE


def build_in_maps(update_location, feature_map, weight, gamma, beta,
                  running_mean, running_var):
    fm = np.asarray(feature_map, np.float32)
    loc = np.asarray(update_location).astype(np.int64)
    wt = np.asarray(weight, np.float32)

    fm_pad = np.pad(fm, ((PAD, PAD), (PAD, PAD), (0, 0)))          # [646,646,3]
    # stripes B_T[t=(i,ch), r, c] = fm_pad[r+i, c, ch], r in 0..640 (row 640 pad)
    bt = np.zeros((21, H + 1, WPAD), np.float32)
    for i in range(K):
        for ch in range(CIN):
            bt[i * CIN + ch, 0:H, :] = fm_pad[i:i + H, :, ch]
    bt = bt.astype(ml_dtypes.bfloat16)

    # inactive flag = 1 where no site; indexed by output pixel (r, c) at
    # position c in the 646-pitch row; columns 640..645 stay inactive.
    flag = np.ones((H + 1, WPAD), np.float32)
    flag[loc[:, 0], loc[:, 1]] = 0.0
    flag[:, H:] = 1.0
    flag = flag.astype(ml_dtypes.bfloat16)

    # reordered weights W_re[(j,i,ch), o] = weight[i, j, ch, o]
    w_re = np.ascontiguousarray(
        wt.transpose(1, 0, 2, 3).reshape(147, COUT)).astype(np.float32)

    bcast = lambda v: np.ascontiguousarray(
        np.broadcast_to(np.asarray(v, np.float32)[None, :], (128, COUT)))

    in_maps = []
    for k in range(NCORES):
        r0 = 80 * k
        t6 = np.zeros((128, BROWS, WPAD), ml_dtypes.bfloat16)
        for j in range(6):
            sl = bt[:, r0:r0 + BROWS, :]
            t6[j * 21:(j + 1) * 21, :, :-j or None] = sl[:, :, j:]
        t6[126] = flag[r0:r0 + BROWS]
        t6[127] = np.ones((BROWS, WPAD), ml_dtypes.bfloat16)
        wfull = np.zeros((128, COUT), np.float32)
        wfull[0:126] = w_re[0:126]
        sel126 = np.zeros((128, COUT), np.float32)
        sel126[126] = 1.0
        sel127 = np.zeros((128, COUT), np.float32)
        sel127[127] = 1.0
        in_maps.append({
            "t6": np.ascontiguousarray(t6.reshape(128, NB)),
            "w": wfull,
            "wtail": np.ascontiguousarray(w_re[126:147]),
            "sel126": sel126, "sel127": sel127,
            "gam": bcast(gamma), "bet": bcast(beta),
            "mu": bcast(running_mean), "var": bcast(running_var),
        })
    return in_maps


def kernel(update_location, feature_map, weight, gamma, beta, running_mean,
           running_var):
    from concourse.bass_utils import run_bass_kernel_spmd

    in_maps = build_in_maps(update_location, feature_map, weight, gamma, beta,
                            running_mean, running_var)
    nc = _get_nc()
    res = run_bass_kernel_spmd(nc, in_maps, core_ids=list(range(NCORES)))
    # per-core out is [64, PROWS*320] f32 (ch-major); assemble [319, 319, 64]
    parts = []
    for k in range(NCORES):
        o = res.results[k]["out"].reshape(COUT, PROWS, 320)
        parts.append(o.transpose(1, 2, 0)[:, :QCOLS, :])
    out = np.concatenate(parts, axis=0)[:QCOLS]
    return np.ascontiguousarray(out).astype(np.float32)


# revision 22
# speedup vs baseline: 1.6219x; 1.1778x over previous
"""Trainium2 Bass kernel for nn_AsynBaseStem (sparse 7x7 conv + BN + ReLU +
scatter + 3x3/2 maxpool), 8-core data-parallel over output row bands.

Architecture (per core, fully dense, no indirect DMA):
  - Host prebuilds a [128, 81*646] bf16 operand table T6 per core:
      rows 0..125  : (j,i,ch) j<6 -> fm_pad[r+i, c+j, ch]  (column-shifted planar stripes)
      row  126     : inactive flag (1.0 where pixel has no site, else 0.0)
      row  127     : ones (bias row)
  - Dense conv at every pixel via 2 accumulating matmuls (K=128 main + K=21
    tail read from T6 rows 0..20 at col offset +6). The flag row adds -1e9 to
    inactive pixels (masking), the ones row adds the BN bias.
  - PSUM eviction fuses the column max-pool (DVE even/odd max + ACT third-col
    copy), then a row ring-buffer completes the 3x3/2 max pool.
  - Final ReLU folded into the row pool; one cast-DMA writes [64, p*320] f32;
    the host transposes to [p, q, ch] during unsharding.

kernel(**inputs) takes FULL unsharded inputs, returns [319, 319, 64] f32.
"""
import numpy as np
import ml_dtypes
from contextlib import ExitStack

H = W = 640
CIN, COUT = 3, 64
K, PAD = 7, 3
NCORES = 8
BROWS = 81            # dense rows per core band
WPAD = W + 2 * PAD    # 646
NB = BROWS * WPAD     # T6 free size per core
PROWS = 40            # pooled rows per core (core 7: 39 valid)
QCOLS = 319
BN_EPS = 1e-5
NEG = -1.0e9


def _build_bass():
    import concourse.bass as bass
    import concourse.mybir as mybir
    import concourse.tile as tile
    from concourse import bacc

    fp32 = mybir.dt.float32
    bf16 = mybir.dt.bfloat16

    nc = bacc.Bacc()
    t6_ext = nc.declare_dram_parameter("t6", [128, NB], bf16, isOutput=False)
    w_ext = nc.declare_dram_parameter("w", [128, COUT], fp32, isOutput=False)
    wt_ext = nc.declare_dram_parameter("wtail", [21, COUT], fp32, isOutput=False)
    s126_ext = nc.declare_dram_parameter("sel126", [128, COUT], fp32, isOutput=False)
    s127_ext = nc.declare_dram_parameter("sel127", [128, COUT], fp32, isOutput=False)
    gam_ext = nc.declare_dram_parameter("gam", [128, COUT], fp32, isOutput=False)
    bet_ext = nc.declare_dram_parameter("bet", [128, COUT], fp32, isOutput=False)
    mu_ext = nc.declare_dram_parameter("mu", [128, COUT], fp32, isOutput=False)
    var_ext = nc.declare_dram_parameter("var", [128, COUT], fp32, isOutput=False)
    out_ext = nc.declare_dram_parameter("out", [COUT, PROWS * 320], fp32, isOutput=True)

    with ExitStack() as ctx:
        tc = ctx.enter_context(tile.TileContext(nc))
        cpool = ctx.enter_context(tc.tile_pool(name="const", bufs=1))
        rowp = ctx.enter_context(tc.tile_pool(name="rows", bufs=12))
        ringp = ctx.enter_context(tc.tile_pool(name="ring", bufs=1))
        psp = ctx.enter_context(tc.tile_pool(name="ps", bufs=8, space="PSUM"))

        # ---- weight prep: lhsT A [128, 64] (W'[0:126] + flag row + bias row),
        #      lhsT B [21, 64] (W'[126:147]); W' = W * inv, inv = gamma*rsqrt(var+eps)
        wa_f = cpool.tile([128, COUT], fp32)
        nc.sync.dma_start(wa_f[:], w_ext[:])
        wb_f = cpool.tile([21, COUT], fp32)
        nc.sync.dma_start(wb_f[:], wt_ext[:])
        s126 = cpool.tile([128, COUT], fp32)
        nc.sync.dma_start(s126[:], s126_ext[:])
        s127 = cpool.tile([128, COUT], fp32)
        nc.sync.dma_start(s127[:], s127_ext[:])
        gam = cpool.tile([128, COUT], fp32)
        nc.sync.dma_start(gam[:], gam_ext[:])
        bet = cpool.tile([128, COUT], fp32)
        nc.sync.dma_start(bet[:], bet_ext[:])
        mu = cpool.tile([128, COUT], fp32)
        nc.sync.dma_start(mu[:], mu_ext[:])
        var = cpool.tile([128, COUT], fp32)
        nc.sync.dma_start(var[:], var_ext[:])

        # ---- big operand table: chunked load AFTER the small parameter DMAs
        # (HWDGE is FIFO per queue) so weight prep and the first conv rows
        # don't wait for the full 13.4MB stream
        t6 = cpool.tile([128, NB], bf16)
        for ck in range(9):
            sl = slice(ck * 9 * WPAD, (ck + 1) * 9 * WPAD)
            nc.sync.dma_start(t6[:, sl], t6_ext[:, sl])

        inv = cpool.tile([128, COUT], fp32)
        nc.vector.tensor_scalar_add(inv[:], var[:], BN_EPS)
        nc.scalar.activation(inv[:], inv[:], mybir.ActivationFunctionType.Sqrt)
        nc.vector.reciprocal(inv[:], inv[:])
        nc.vector.tensor_mul(inv[:], inv[:], gam[:])
        # bias' = beta - mu*inv ; flagC = NEG - bias'
        biasv = cpool.tile([128, COUT], fp32)
        nc.vector.tensor_mul(biasv[:], mu[:], inv[:])
        nc.vector.tensor_sub(biasv[:], bet[:], biasv[:])
        flagc = cpool.tile([128, COUT], fp32)
        nc.vector.tensor_scalar(
            out=flagc[:], in0=biasv[:], scalar1=-1.0, scalar2=NEG,
            op0=mybir.AluOpType.mult, op1=mybir.AluOpType.add,
        )

        # lhsA = wa*inv (rows 126/127 are zero in wa) + sel126*flagC + sel127*bias'
        acc = cpool.tile([128, COUT], fp32)
        nc.vector.tensor_mul(acc[:], wa_f[:], inv[:])
        t1 = cpool.tile([128, COUT], fp32)
        nc.vector.tensor_mul(t1[:], s126[:], flagc[:])
        nc.vector.tensor_add(acc[:], acc[:], t1[:])
        nc.vector.tensor_mul(t1[:], s127[:], biasv[:])
        lhsA = cpool.tile([128, COUT], bf16)
        nc.vector.tensor_add(lhsA[:], acc[:], t1[:])
        lhsB = cpool.tile([21, COUT], bf16)
        nc.vector.tensor_mul(lhsB[:], wb_f[:], inv[0:21, :])

        # ---- pooled accumulator [64, PROWS, 320] bf16 and row ring ----
        pooled = ringp.tile([COUT, PROWS * 320], bf16)
        mring = ringp.tile([COUT, 8 * 320], bf16)  # m rows modulo 8

        # halves: (c0, width, qbase, nq)
        halves = ((0, 322, 0, 160), (320, 320, 160, 159))

        for r in range(BROWS):
            mrow = mring[:, (r % 8) * 320:(r % 8) * 320 + 320]
            for c0, wdt, qb, nq in halves:
                ps = psp.tile([COUT, 322], fp32, tag="convps")
                x = r * WPAD + c0
                nc.tensor.matmul(ps[:, 0:wdt], lhsA[:], t6[0:128, x:x + wdt],
                                 start=True, stop=False)
                nc.tensor.matmul(ps[:, 0:wdt], lhsB[:], t6[0:21, x + 6:x + 6 + wdt],
                                 start=False, stop=True)
                # col-pool m[q] = max(d[2q], d[2q+1], d[2q+2]):
                # ACT stages even cols (ev[q] = d[2q], also provides d[2q+2] = ev[q+1]);
                # DVE: t = max(ev, d-odd[psum]); m = max(t, ev-shifted)
                ev = rowp.tile([COUT, 161], bf16, tag="ev")
                nc.scalar.copy(ev[:, 0:nq + 1], ps[:, 0:2 * (nq + 1):2])
                trow = rowp.tile([COUT, 161], bf16, tag="trow")
                nc.vector.tensor_tensor(
                    out=trow[:, 0:nq],
                    in0=ev[:, 0:nq], in1=ps[:, 1:2 * nq:2],
                    op=mybir.AluOpType.max)
                nc.vector.tensor_tensor(
                    out=mrow[:, qb:qb + nq], in0=trow[:, 0:nq], in1=ev[:, 1:nq + 1],
                    op=mybir.AluOpType.max)
            # row pool (+final relu): pooled[p] = max(m[2p], m[2p+1], m[2p+2], 0)
            if r >= 2 and r % 2 == 0:
                p = (r - 2) // 2
                m0 = mring[:, ((r - 2) % 8) * 320:((r - 2) % 8) * 320 + 320]
                m1 = mring[:, ((r - 1) % 8) * 320:((r - 1) % 8) * 320 + 320]
                m2 = mring[:, (r % 8) * 320:(r % 8) * 320 + 320]
                s01 = rowp.tile([COUT, 320], bf16, tag="s01")
                nc.vector.tensor_tensor(out=s01[:], in0=m0[:], in1=m1[:],
                                        op=mybir.AluOpType.max)
                po = pooled[:, p * 320:(p + 1) * 320]
                nc.vector.scalar_tensor_tensor(
                    out=po[:], in0=s01[:], scalar=0.0, in1=m2[:],
                    op0=mybir.AluOpType.max, op1=mybir.AluOpType.max)
                # stream pooled rows out in chunks of 10 (cast bf16 -> f32);
                # host does the final [p,q,ch] transpose during unsharding
                if p % 10 == 9:
                    pc = p // 10
                    nc.gpsimd.dma_start(
                        out_ext[:, pc * 3200:(pc + 1) * 3200],
                        pooled[:, pc * 3200:(pc + 1) * 3200])



    nc.finalize()
    return nc


_NC_CACHE = None


def _get_nc():
    global _NC_CACHE
    if _NC_CACHE is None:
        _NC_CACHE = _build_bass()
    return _NC_CACHE


def build_in_maps(update_location, feature_map, weight, gamma, beta,
                  running_mean, running_var):
    fm = np.asarray(feature_map, np.float32)
    loc = np.asarray(update_location).astype(np.int64)
    wt = np.asarray(weight, np.float32)

    fm_pad = np.pad(fm, ((PAD, PAD), (PAD, PAD), (0, 0)))          # [646,646,3]
    # stripes B_T[t=(i,ch), r, c] = fm_pad[r+i, c, ch], r in 0..640 (row 640 pad)
    bt = np.zeros((21, H + 1, WPAD), np.float32)
    for i in range(K):
        for ch in range(CIN):
            bt[i * CIN + ch, 0:H, :] = fm_pad[i:i + H, :, ch]
    bt = bt.astype(ml_dtypes.bfloat16)

    # inactive flag = 1 where no site; indexed by output pixel (r, c) at
    # position c in the 646-pitch row; columns 640..645 stay inactive.
    flag = np.ones((H + 1, WPAD), np.float32)
    flag[loc[:, 0], loc[:, 1]] = 0.0
    flag[:, H:] = 1.0
    flag = flag.astype(ml_dtypes.bfloat16)

    # reordered weights W_re[(j,i,ch), o] = weight[i, j, ch, o]
    w_re = np.ascontiguousarray(
        wt.transpose(1, 0, 2, 3).reshape(147, COUT)).astype(np.float32)

    bcast = lambda v: np.ascontiguousarray(
        np.broadcast_to(np.asarray(v, np.float32)[None, :], (128, COUT)))

    in_maps = []
    for k in range(NCORES):
        r0 = 80 * k
        t6 = np.zeros((128, BROWS, WPAD), ml_dtypes.bfloat16)
        for j in range(6):
            sl = bt[:, r0:r0 + BROWS, :]
            t6[j * 21:(j + 1) * 21, :, :-j or None] = sl[:, :, j:]
        t6[126] = flag[r0:r0 + BROWS]
        t6[127] = np.ones((BROWS, WPAD), ml_dtypes.bfloat16)
        wfull = np.zeros((128, COUT), np.float32)
        wfull[0:126] = w_re[0:126]
        sel126 = np.zeros((128, COUT), np.float32)
        sel126[126] = 1.0
        sel127 = np.zeros((128, COUT), np.float32)
        sel127[127] = 1.0
        in_maps.append({
            "t6": np.ascontiguousarray(t6.reshape(128, NB)),
            "w": wfull,
            "wtail": np.ascontiguousarray(w_re[126:147]),
            "sel126": sel126, "sel127": sel127,
            "gam": bcast(gamma), "bet": bcast(beta),
            "mu": bcast(running_mean), "var": bcast(running_var),
        })
    return in_maps


def kernel(update_location, feature_map, weight, gamma, beta, running_mean,
           running_var):
    from concourse.bass_utils import run_bass_kernel_spmd

    in_maps = build_in_maps(update_location, feature_map, weight, gamma, beta,
                            running_mean, running_var)
    nc = _get_nc()
    res = run_bass_kernel_spmd(nc, in_maps, core_ids=list(range(NCORES)))
    # per-core out is [64, PROWS*320] f32 (ch-major); assemble [319, 319, 64]
    parts = []
    for k in range(NCORES):
        o = res.results[k]["out"].reshape(COUT, PROWS, 320)
        parts.append(o.transpose(1, 2, 0)[:, :QCOLS, :])
    out = np.concatenate(parts, axis=0)[:QCOLS]
    return np.ascontiguousarray(out).astype(np.float32)


# revision 26
# speedup vs baseline: 1.7888x; 1.1029x over previous
"""Trainium2 Bass kernel for nn_AsynBaseStem (sparse 7x7 conv + BN + ReLU +
scatter + 3x3/2 maxpool), 8-core data-parallel over output row bands.

Architecture (per core, fully dense, no indirect DMA):
  - Host prebuilds a [128, 81*646] bf16 operand table T6 per core:
      rows 0..125  : (j,i,ch) j<6 -> fm_pad[r+i, c+j, ch]  (column-shifted planar stripes)
      row  126     : inactive flag (1.0 where pixel has no site, else 0.0)
      row  127     : ones (bias row)
  - Dense conv at every pixel via 2 accumulating matmuls (K=128 main + K=21
    tail read from T6 rows 0..20 at col offset +6). The flag row adds -1e9 to
    inactive pixels (masking), the ones row adds the BN bias.
  - PSUM eviction fuses the column max-pool (DVE even/odd max + ACT third-col
    copy), then a row ring-buffer completes the 3x3/2 max pool.
  - Final ReLU folded into the row pool; one cast-DMA writes [64, p*320] f32;
    the host transposes to [p, q, ch] during unsharding.

kernel(**inputs) takes FULL unsharded inputs, returns [319, 319, 64] f32.
"""
import numpy as np
import ml_dtypes
from contextlib import ExitStack

H = W = 640
CIN, COUT = 3, 64
K, PAD = 7, 3
NCORES = 8
BROWS = 81            # dense rows per core band
WPAD = W + 2 * PAD    # 646
NB = BROWS * WPAD     # T6 free size per core
NBP = NB + 8          # +pad so the tail matmul window (x+6) stays in bounds
PROWS = 40            # pooled rows per core (core 7: 39 valid)
QCOLS = 319
BN_EPS = 1e-5
NEG = -1.0e9


def _build_bass():
    import concourse.bass as bass
    import concourse.mybir as mybir
    import concourse.tile as tile
    from concourse import bacc

    fp32 = mybir.dt.float32
    bf16 = mybir.dt.bfloat16

    nc = bacc.Bacc()
    t6_ext = nc.declare_dram_parameter("t6", [128, NBP], bf16, isOutput=False)
    w_ext = nc.declare_dram_parameter("w", [128, COUT], fp32, isOutput=False)
    wt_ext = nc.declare_dram_parameter("wtail", [21, COUT], fp32, isOutput=False)
    s126_ext = nc.declare_dram_parameter("sel126", [128, COUT], fp32, isOutput=False)
    s127_ext = nc.declare_dram_parameter("sel127", [128, COUT], fp32, isOutput=False)
    gam_ext = nc.declare_dram_parameter("gam", [128, COUT], fp32, isOutput=False)
    bet_ext = nc.declare_dram_parameter("bet", [128, COUT], fp32, isOutput=False)
    mu_ext = nc.declare_dram_parameter("mu", [128, COUT], fp32, isOutput=False)
    var_ext = nc.declare_dram_parameter("var", [128, COUT], fp32, isOutput=False)
    out_ext = nc.declare_dram_parameter("out", [COUT, PROWS * 320], fp32, isOutput=True)

    with ExitStack() as ctx:
        tc = ctx.enter_context(tile.TileContext(nc))
        cpool = ctx.enter_context(tc.tile_pool(name="const", bufs=1))
        rowp = ctx.enter_context(tc.tile_pool(name="rows", bufs=12))
        ringp = ctx.enter_context(tc.tile_pool(name="ring", bufs=1))
        psp = ctx.enter_context(tc.tile_pool(name="ps", bufs=8, space="PSUM"))

        # ---- weight prep: lhsT A [128, 64] (W'[0:126] + flag row + bias row),
        #      lhsT B [21, 64] (W'[126:147]); W' = W * inv, inv = gamma*rsqrt(var+eps)
        wa_f = cpool.tile([128, COUT], fp32)
        nc.sync.dma_start(wa_f[:], w_ext[:])
        wb_f = cpool.tile([21, COUT], fp32)
        nc.sync.dma_start(wb_f[:], wt_ext[:])
        s126 = cpool.tile([128, COUT], fp32)
        nc.sync.dma_start(s126[:], s126_ext[:])
        s127 = cpool.tile([128, COUT], fp32)
        nc.sync.dma_start(s127[:], s127_ext[:])
        gam = cpool.tile([128, COUT], fp32)
        nc.sync.dma_start(gam[:], gam_ext[:])
        bet = cpool.tile([128, COUT], fp32)
        nc.sync.dma_start(bet[:], bet_ext[:])
        mu = cpool.tile([128, COUT], fp32)
        nc.sync.dma_start(mu[:], mu_ext[:])
        var = cpool.tile([128, COUT], fp32)
        nc.sync.dma_start(var[:], var_ext[:])

        # ---- big operand table: chunked load AFTER the small parameter DMAs
        # (HWDGE is FIFO per queue) so weight prep and the first conv rows
        # don't wait for the full 13.4MB stream
        t6 = cpool.tile([128, NBP], bf16)
        for ck in range(9):
            sl = slice(ck * 9 * WPAD, (ck + 1) * 9 * WPAD if ck < 8 else NBP)
            nc.sync.dma_start(t6[:, sl], t6_ext[:, sl])

        inv = cpool.tile([128, COUT], fp32)
        nc.vector.tensor_scalar_add(inv[:], var[:], BN_EPS)
        nc.scalar.activation(inv[:], inv[:], mybir.ActivationFunctionType.Sqrt)
        nc.vector.reciprocal(inv[:], inv[:])
        nc.vector.tensor_mul(inv[:], inv[:], gam[:])
        # bias' = beta - mu*inv ; flagC = NEG - bias'
        biasv = cpool.tile([128, COUT], fp32)
        nc.vector.tensor_mul(biasv[:], mu[:], inv[:])
        nc.vector.tensor_sub(biasv[:], bet[:], biasv[:])
        flagc = cpool.tile([128, COUT], fp32)
        nc.vector.tensor_scalar(
            out=flagc[:], in0=biasv[:], scalar1=-1.0, scalar2=NEG,
            op0=mybir.AluOpType.mult, op1=mybir.AluOpType.add,
        )

        # lhsA = wa*inv (rows 126/127 are zero in wa) + sel126*flagC + sel127*bias'
        acc = cpool.tile([128, COUT], fp32)
        nc.vector.tensor_mul(acc[:], wa_f[:], inv[:])
        t1 = cpool.tile([128, COUT], fp32)
        nc.vector.tensor_mul(t1[:], s126[:], flagc[:])
        nc.vector.tensor_add(acc[:], acc[:], t1[:])
        nc.vector.tensor_mul(t1[:], s127[:], biasv[:])
        lhsA = cpool.tile([128, COUT], bf16)
        nc.vector.tensor_add(lhsA[:], acc[:], t1[:])
        lhsB = cpool.tile([21, COUT], bf16)
        nc.vector.tensor_mul(lhsB[:], wb_f[:], inv[0:21, :])

        # ---- pooled accumulator [64, PROWS, 320] bf16 and row ring ----
        pooled = ringp.tile([COUT, PROWS * 320], bf16)
        mring = ringp.tile([COUT, 8 * 320], bf16)  # m rows modulo 8

        # Continuous-pixel-space conv: N=512 matmul tiles over x in [0, NB).
        # Row-boundary/pad pixels carry flag=1 -> -1e9, so the pool ignores
        # them. Per-row ev (even cols) and t (pair-max) staging buffers absorb
        # tile fragments; a full-row m then feeds the row pool.
        NT = (NB + 511) // 512
        evrow = {}
        trow = {}

        def finish_row(r):
            mrow = mring[:, (r % 8) * 320:(r % 8) * 320 + 320]
            nc.vector.tensor_tensor(
                out=mrow[:], in0=trow[r][:, 0:320], in1=evrow[r][:, 1:321],
                op=mybir.AluOpType.max)
            del evrow[r], trow[r]
            if r >= 2 and r % 2 == 0:
                p = (r - 2) // 2
                m0 = mring[:, ((r - 2) % 8) * 320:((r - 2) % 8) * 320 + 320]
                m1 = mring[:, ((r - 1) % 8) * 320:((r - 1) % 8) * 320 + 320]
                s01 = rowp.tile([COUT, 320], bf16, tag="s01")
                nc.vector.tensor_tensor(out=s01[:], in0=m0[:], in1=m1[:],
                                        op=mybir.AluOpType.max)
                po = pooled[:, p * 320:(p + 1) * 320]
                nc.vector.scalar_tensor_tensor(
                    out=po[:], in0=s01[:], scalar=0.0, in1=mrow[:],
                    op0=mybir.AluOpType.max, op1=mybir.AluOpType.max)
                # stream pooled rows out in chunks of 10 (cast bf16 -> f32);
                # host does the final [p,q,ch] transpose during unsharding
                if p % 10 == 9:
                    pc = p // 10
                    nc.gpsimd.dma_start(
                        out_ext[:, pc * 3200:(pc + 1) * 3200],
                        pooled[:, pc * 3200:(pc + 1) * 3200])

        for k in range(NT):
            xa = 512 * k
            xb = min(xa + 512, NB)
            wdt = xb - xa
            ps = psp.tile([COUT, 512], fp32, tag="convps")
            nc.tensor.matmul(ps[:, 0:wdt], lhsA[:], t6[0:128, xa:xb],
                             start=True, stop=False)
            nc.tensor.matmul(ps[:, 0:wdt], lhsB[:], t6[0:21, xa + 6:xb + 6],
                             start=False, stop=True)
            for r in range(xa // WPAD, (xb - 1) // WPAD + 1):
                ca = max(xa, r * WPAD) - r * WPAD     # even
                cb = min(xb, r * WPAD + WPAD) - r * WPAD  # even
                if r not in evrow:
                    evrow[r] = rowp.tile([COUT, 324], bf16, tag="evrow", name=f"evrow{r}")
                    trow[r] = rowp.tile([COUT, 324], bf16, tag="trowb", name=f"trowb{r}")
                ne = (cb - ca) // 2
                p0 = r * WPAD + ca - xa               # psum-local offset
                nc.scalar.copy(evrow[r][:, ca // 2:ca // 2 + ne],
                               ps[:, p0:p0 + 2 * ne:2])
                nc.vector.tensor_tensor(
                    out=trow[r][:, ca // 2:ca // 2 + ne],
                    in0=evrow[r][:, ca // 2:ca // 2 + ne],
                    in1=ps[:, p0 + 1:p0 + 2 * ne:2],
                    op=mybir.AluOpType.max)
                if cb == WPAD:
                    finish_row(r)



    nc.finalize()
    return nc


_NC_CACHE = None


def _get_nc():
    global _NC_CACHE
    if _NC_CACHE is None:
        _NC_CACHE = _build_bass()
    return _NC_CACHE


def build_in_maps(update_location, feature_map, weight, gamma, beta,
                  running_mean, running_var):
    fm = np.asarray(feature_map, np.float32)
    loc = np.asarray(update_location).astype(np.int64)
    wt = np.asarray(weight, np.float32)

    fm_pad = np.pad(fm, ((PAD, PAD), (PAD, PAD), (0, 0)))          # [646,646,3]
    # stripes B_T[t=(i,ch), r, c] = fm_pad[r+i, c, ch], r in 0..640 (row 640 pad)
    bt = np.zeros((21, H + 1, WPAD), np.float32)
    for i in range(K):
        for ch in range(CIN):
            bt[i * CIN + ch, 0:H, :] = fm_pad[i:i + H, :, ch]
    bt = bt.astype(ml_dtypes.bfloat16)

    # inactive flag = 1 where no site; indexed by output pixel (r, c) at
    # position c in the 646-pitch row; columns 640..645 stay inactive.
    flag = np.ones((H + 1, WPAD), np.float32)
    flag[loc[:, 0], loc[:, 1]] = 0.0
    flag[:, H:] = 1.0
    flag = flag.astype(ml_dtypes.bfloat16)

    # reordered weights W_re[(j,i,ch), o] = weight[i, j, ch, o]
    w_re = np.ascontiguousarray(
        wt.transpose(1, 0, 2, 3).reshape(147, COUT)).astype(np.float32)

    bcast = lambda v: np.ascontiguousarray(
        np.broadcast_to(np.asarray(v, np.float32)[None, :], (128, COUT)))

    in_maps = []
    for k in range(NCORES):
        r0 = 80 * k
        t6 = np.zeros((128, BROWS, WPAD), ml_dtypes.bfloat16)
        for j in range(6):
            sl = bt[:, r0:r0 + BROWS, :]
            t6[j * 21:(j + 1) * 21, :, :-j or None] = sl[:, :, j:]
        t6[126] = flag[r0:r0 + BROWS]
        t6[127] = np.ones((BROWS, WPAD), ml_dtypes.bfloat16)
        wfull = np.zeros((128, COUT), np.float32)
        wfull[0:126] = w_re[0:126]
        sel126 = np.zeros((128, COUT), np.float32)
        sel126[126] = 1.0
        sel127 = np.zeros((128, COUT), np.float32)
        sel127[127] = 1.0
        t6p = np.zeros((128, NBP), ml_dtypes.bfloat16)
        t6p[:, :NB] = t6.reshape(128, NB)
        in_maps.append({
            "t6": t6p,
            "w": wfull,
            "wtail": np.ascontiguousarray(w_re[126:147]),
            "sel126": sel126, "sel127": sel127,
            "gam": bcast(gamma), "bet": bcast(beta),
            "mu": bcast(running_mean), "var": bcast(running_var),
        })
    return in_maps


def kernel(update_location, feature_map, weight, gamma, beta, running_mean,
           running_var):
    from concourse.bass_utils import run_bass_kernel_spmd

    in_maps = build_in_maps(update_location, feature_map, weight, gamma, beta,
                            running_mean, running_var)
    nc = _get_nc()
    res = run_bass_kernel_spmd(nc, in_maps, core_ids=list(range(NCORES)))
    # per-core out is [64, PROWS*320] f32 (ch-major); assemble [319, 319, 64]
    parts = []
    for k in range(NCORES):
        o = res.results[k]["out"].reshape(COUT, PROWS, 320)
        parts.append(o.transpose(1, 2, 0)[:, :QCOLS, :])
    out = np.concatenate(parts, axis=0)[:QCOLS]
    return np.ascontiguousarray(out).astype(np.float32)


# revision 27
# speedup vs baseline: 1.8569x; 1.0381x over previous
"""Trainium2 Bass kernel for nn_AsynBaseStem (sparse 7x7 conv + BN + ReLU +
scatter + 3x3/2 maxpool), 8-core data-parallel over output row bands.

Architecture (per core, fully dense, no indirect DMA):
  - Host prebuilds a [128, 81*646] bf16 operand table T6 per core:
      rows 0..125  : (j,i,ch) j<6 -> fm_pad[r+i, c+j, ch]  (column-shifted planar stripes)
      row  126     : inactive flag (1.0 where pixel has no site, else 0.0)
      row  127     : ones (bias row)
  - Dense conv at every pixel via 2 accumulating matmuls (K=128 main + K=21
    tail read from T6 rows 0..20 at col offset +6). The flag row adds -1e9 to
    inactive pixels (masking), the ones row adds the BN bias.
  - PSUM eviction fuses the column max-pool (DVE even/odd max + ACT third-col
    copy), then a row ring-buffer completes the 3x3/2 max pool.
  - Final ReLU folded into the row pool; one cast-DMA writes [64, p*320] f32;
    the host transposes to [p, q, ch] during unsharding.

kernel(**inputs) takes FULL unsharded inputs, returns [319, 319, 64] f32.
"""
import numpy as np
import ml_dtypes
from contextlib import ExitStack

H = W = 640
CIN, COUT = 3, 64
K, PAD = 7, 3
NCORES = 8
BROWS = 81            # dense rows per core band
WPAD = W + 2 * PAD    # 646
NB = BROWS * WPAD     # T6 free size per core
NBP = NB + 8          # +pad so the tail matmul window (x+6) stays in bounds
PROWS = 40            # pooled rows per core (core 7: 39 valid)
QCOLS = 319
BN_EPS = 1e-5
NEG = -1.0e9


def _build_bass():
    import concourse.bass as bass
    import concourse.mybir as mybir
    import concourse.tile as tile
    from concourse import bacc

    fp32 = mybir.dt.float32
    bf16 = mybir.dt.bfloat16

    nc = bacc.Bacc()
    t6_ext = nc.declare_dram_parameter("t6", [128, NBP], bf16, isOutput=False)
    # packed params: [w | wtail(pad128) | sel126 | sel127 | gam | bet | mu | var]
    par_ext = nc.declare_dram_parameter("par", [128, 8 * COUT], fp32, isOutput=False)
    out_ext = nc.declare_dram_parameter("out", [COUT, PROWS * 320], fp32, isOutput=True)

    with ExitStack() as ctx:
        tc = ctx.enter_context(tile.TileContext(nc))
        cpool = ctx.enter_context(tc.tile_pool(name="const", bufs=1))
        rowp = ctx.enter_context(tc.tile_pool(name="rows", bufs=12))
        ringp = ctx.enter_context(tc.tile_pool(name="ring", bufs=1))
        psp = ctx.enter_context(tc.tile_pool(name="ps", bufs=8, space="PSUM"))

        # ---- weight prep: lhsT A [128, 64] (W'[0:126] + flag row + bias row),
        #      lhsT B [21, 64] (W'[126:147]); W' = W * inv, inv = gamma*rsqrt(var+eps)
        par = cpool.tile([128, 8 * COUT], fp32)
        nc.sync.dma_start(par[:], par_ext[:])
        C = COUT
        wa_f = par[:, 0:C]
        wb_f = par[0:21, C:C + C]
        s126 = par[:, 2 * C:3 * C]
        s127 = par[:, 3 * C:4 * C]
        gam = par[:, 4 * C:5 * C]
        bet = par[:, 5 * C:6 * C]
        mu = par[:, 6 * C:7 * C]
        var = par[:, 7 * C:8 * C]

        # ---- big operand table: chunked load AFTER the small parameter DMAs
        # (HWDGE is FIFO per queue) so weight prep and the first conv rows
        # don't wait for the full 13.4MB stream
        t6 = cpool.tile([128, NBP], bf16)
        for ck in range(9):
            sl = slice(ck * 9 * WPAD, (ck + 1) * 9 * WPAD if ck < 8 else NBP)
            nc.sync.dma_start(t6[:, sl], t6_ext[:, sl])

        inv = cpool.tile([128, COUT], fp32)
        nc.vector.tensor_scalar_add(inv[:], var, BN_EPS)
        nc.scalar.activation(inv[:], inv[:], mybir.ActivationFunctionType.Sqrt)
        nc.vector.reciprocal(inv[:], inv[:])
        nc.vector.tensor_mul(inv[:], inv[:], gam)
        # bias' = beta - mu*inv ; flagC = NEG - bias'
        biasv = cpool.tile([128, COUT], fp32)
        nc.vector.tensor_mul(biasv[:], mu, inv[:])
        nc.vector.tensor_sub(biasv[:], bet, biasv[:])
        flagc = cpool.tile([128, COUT], fp32)
        nc.vector.tensor_scalar(
            out=flagc[:], in0=biasv[:], scalar1=-1.0, scalar2=NEG,
            op0=mybir.AluOpType.mult, op1=mybir.AluOpType.add,
        )

        # lhsA = wa*inv (rows 126/127 are zero in wa) + sel126*flagC + sel127*bias'
        acc = cpool.tile([128, COUT], fp32)
        nc.vector.tensor_mul(acc[:], wa_f, inv[:])
        t1 = cpool.tile([128, COUT], fp32)
        nc.vector.tensor_mul(t1[:], s126, flagc[:])
        nc.vector.tensor_add(acc[:], acc[:], t1[:])
        nc.vector.tensor_mul(t1[:], s127, biasv[:])
        lhsA = cpool.tile([128, COUT], bf16)
        nc.vector.tensor_add(lhsA[:], acc[:], t1[:])
        lhsB = cpool.tile([21, COUT], bf16)
        nc.vector.tensor_mul(lhsB[:], wb_f, inv[0:21, :])

        # ---- pooled accumulator [64, PROWS, 320] bf16 and row ring ----
        pooled = ringp.tile([COUT, PROWS * 320], bf16)
        mring = ringp.tile([COUT, 8 * 320], bf16)  # m rows modulo 8

        # Continuous-pixel-space conv: N=512 matmul tiles over x in [0, NB).
        # Row-boundary/pad pixels carry flag=1 -> -1e9, so the pool ignores
        # them. Per-row ev (even cols) and t (pair-max) staging buffers absorb
        # tile fragments; a full-row m then feeds the row pool.
        NT = (NB + 511) // 512
        evrow = {}
        trow = {}

        def finish_row(r):
            mrow = mring[:, (r % 8) * 320:(r % 8) * 320 + 320]
            nc.vector.tensor_tensor(
                out=mrow[:], in0=trow[r][:, 0:320], in1=evrow[r][:, 1:321],
                op=mybir.AluOpType.max)
            del evrow[r], trow[r]
            if r >= 2 and r % 2 == 0:
                p = (r - 2) // 2
                m0 = mring[:, ((r - 2) % 8) * 320:((r - 2) % 8) * 320 + 320]
                m1 = mring[:, ((r - 1) % 8) * 320:((r - 1) % 8) * 320 + 320]
                s01 = rowp.tile([COUT, 320], bf16, tag="s01")
                nc.vector.tensor_tensor(out=s01[:], in0=m0[:], in1=m1[:],
                                        op=mybir.AluOpType.max)
                po = pooled[:, p * 320:(p + 1) * 320]
                nc.vector.scalar_tensor_tensor(
                    out=po[:], in0=s01[:], scalar=0.0, in1=mrow[:],
                    op0=mybir.AluOpType.max, op1=mybir.AluOpType.max)
                # stream pooled rows out in chunks of 10 (cast bf16 -> f32);
                # host does the final [p,q,ch] transpose during unsharding
                if p % 5 == 4:
                    pc = p // 5
                    nc.gpsimd.dma_start(
                        out_ext[:, pc * 1600:(pc + 1) * 1600],
                        pooled[:, pc * 1600:(pc + 1) * 1600])

        for k in range(NT):
            xa = 512 * k
            xb = min(xa + 512, NB)
            wdt = xb - xa
            ps = psp.tile([COUT, 512], fp32, tag="convps")
            nc.tensor.matmul(ps[:, 0:wdt], lhsA[:], t6[0:128, xa:xb],
                             start=True, stop=False)
            nc.tensor.matmul(ps[:, 0:wdt], lhsB[:], t6[0:21, xa + 6:xb + 6],
                             start=False, stop=True)
            for r in range(xa // WPAD, (xb - 1) // WPAD + 1):
                ca = max(xa, r * WPAD) - r * WPAD     # even
                cb = min(xb, r * WPAD + WPAD) - r * WPAD  # even
                if r not in evrow:
                    evrow[r] = rowp.tile([COUT, 324], bf16, tag="evrow", name=f"evrow{r}")
                    trow[r] = rowp.tile([COUT, 324], bf16, tag="trowb", name=f"trowb{r}")
                ne = (cb - ca) // 2
                p0 = r * WPAD + ca - xa               # psum-local offset
                nc.scalar.copy(evrow[r][:, ca // 2:ca // 2 + ne],
                               ps[:, p0:p0 + 2 * ne:2])
                nc.vector.tensor_tensor(
                    out=trow[r][:, ca // 2:ca // 2 + ne],
                    in0=evrow[r][:, ca // 2:ca // 2 + ne],
                    in1=ps[:, p0 + 1:p0 + 2 * ne:2],
                    op=mybir.AluOpType.max)
                if cb == WPAD:
                    finish_row(r)



    nc.finalize()
    return nc


_NC_CACHE = None


def _get_nc():
    global _NC_CACHE
    if _NC_CACHE is None:
        _NC_CACHE = _build_bass()
    return _NC_CACHE


def build_in_maps(update_location, feature_map, weight, gamma, beta,
                  running_mean, running_var):
    fm = np.asarray(feature_map, np.float32)
    loc = np.asarray(update_location).astype(np.int64)
    wt = np.asarray(weight, np.float32)

    fm_pad = np.pad(fm, ((PAD, PAD), (PAD, PAD), (0, 0)))          # [646,646,3]
    # stripes B_T[t=(i,ch), r, c] = fm_pad[r+i, c, ch], r in 0..640 (row 640 pad)
    bt = np.zeros((21, H + 1, WPAD), np.float32)
    for i in range(K):
        for ch in range(CIN):
            bt[i * CIN + ch, 0:H, :] = fm_pad[i:i + H, :, ch]
    bt = bt.astype(ml_dtypes.bfloat16)

    # inactive flag = 1 where no site; indexed by output pixel (r, c) at
    # position c in the 646-pitch row; columns 640..645 stay inactive.
    flag = np.ones((H + 1, WPAD), np.float32)
    flag[loc[:, 0], loc[:, 1]] = 0.0
    flag[:, H:] = 1.0
    flag = flag.astype(ml_dtypes.bfloat16)

    # reordered weights W_re[(j,i,ch), o] = weight[i, j, ch, o]
    w_re = np.ascontiguousarray(
        wt.transpose(1, 0, 2, 3).reshape(147, COUT)).astype(np.float32)

    bcast = lambda v: np.ascontiguousarray(
        np.broadcast_to(np.asarray(v, np.float32)[None, :], (128, COUT)))

    in_maps = []
    for k in range(NCORES):
        r0 = 80 * k
        t6 = np.zeros((128, BROWS, WPAD), ml_dtypes.bfloat16)
        for j in range(6):
            sl = bt[:, r0:r0 + BROWS, :]
            t6[j * 21:(j + 1) * 21, :, :-j or None] = sl[:, :, j:]
        t6[126] = flag[r0:r0 + BROWS]
        t6[127] = np.ones((BROWS, WPAD), ml_dtypes.bfloat16)
        wfull = np.zeros((128, COUT), np.float32)
        wfull[0:126] = w_re[0:126]
        sel126 = np.zeros((128, COUT), np.float32)
        sel126[126] = 1.0
        sel127 = np.zeros((128, COUT), np.float32)
        sel127[127] = 1.0
        t6p = np.zeros((128, NBP), ml_dtypes.bfloat16)
        t6p[:, :NB] = t6.reshape(128, NB)
        wtail = np.zeros((128, COUT), np.float32)
        wtail[0:21] = w_re[126:147]
        par = np.concatenate([wfull, wtail, sel126, sel127, bcast(gamma),
                              bcast(beta), bcast(running_mean),
                              bcast(running_var)], axis=1)
        in_maps.append({"t6": t6p, "par": np.ascontiguousarray(par)})
    return in_maps


def kernel(update_location, feature_map, weight, gamma, beta, running_mean,
           running_var):
    from concourse.bass_utils import run_bass_kernel_spmd

    in_maps = build_in_maps(update_location, feature_map, weight, gamma, beta,
                            running_mean, running_var)
    nc = _get_nc()
    res = run_bass_kernel_spmd(nc, in_maps, core_ids=list(range(NCORES)))
    # per-core out is [64, PROWS*320] f32 (ch-major); assemble [319, 319, 64]
    parts = []
    for k in range(NCORES):
        o = res.results[k]["out"].reshape(COUT, PROWS, 320)
        parts.append(o.transpose(1, 2, 0)[:, :QCOLS, :])
    out = np.concatenate(parts, axis=0)[:QCOLS]
    return np.ascontiguousarray(out).astype(np.float32)
